# revision 12
# baseline (speedup 1.0000x reference)
"""Trainium2 Bass kernel for a 2-layer GCN (EnhancedHockeyGNN) — v5.

v5 = v4 + natural-tile grouping + cohort-packed gather rows:
  - Groups are the natural 128-node tiles of each core's shard (no bin
    packing). Self-loop edges are dropped from the gather entirely — the
    self contribution is one identity matmul from the locally-resident
    xs tile (xsb / xs2b) into the group's PSUM accumulation.
  - Gather rows are packed contiguously per (cohort, chunk) with shared
    boundary tiles (per-group capacity = cross-core max edge count, no
    per-group 128-rounding): ~14% fewer SWDGE descriptors, which is the
    hard bottleneck (all dynamic-DMA rings share one DMA engine).
  - Layer-2 message tables have the same row layout as layer-1's (both
    are in natural node order), so ONE idx/dloc table set serves both
    layers — no mid-kernel table swap.
  - Gather sub-calls are assigned round-robin across the 4 SWDGE queues
    (not queue=chunk), so all queues start as soon as the first
    AllGather chunks land and stay balanced.
"""
import math

import numpy as np

# ---------------------------------------------------------------- constants
N = 100000
F_IN = 128
H = 128
NC = 8
SHARD = 12544            # multiple of 128; 8 * 12544 = 100352 >= N
NPAD = NC * SHARD
NCHUNK = 4               # AllGather chunks == src buckets (int16 idx limit)
COH = 4                  # groups per gather cohort
NQ = 4                   # SWDGE queues
EPS = 1e-5

_CACHE = {}


def _chunks(n, k):
    k = min(k, n)
    base, rem = n // k, n % k
    out, lo = [], 0
    for i in range(k):
        hi = lo + base + (1 if i < rem else 0)
        out.append((lo, hi))
        lo = hi
    return out


def _wrap_idx16(idx_flat):
    """[n] int16 -> [128, n//16] wrapped (i -> [i%16, i//16]) + replicated."""
    n = idx_flat.shape[0]
    assert n % 16 == 0
    w = idx_flat.reshape(n // 16, 16).T            # [16, cols]
    return np.tile(w, (8, 1)).copy()               # [128, cols]


# ---------------------------------------------------------------- host prep
def _prepare(x, edge_index, game_indices, cfg):
    n, npad, shard, nc = cfg["N"], cfg["NPAD"], cfg["SHARD"], cfg["NC"]
    nchunk, coh, nq = cfg["NCHUNK"], cfg["COH"], cfg["NQ"]
    cap_tiles = cfg.get("NIDX_CAP", 1024) // 128

    src0 = np.asarray(edge_index[0], dtype=np.int64)
    dst0 = np.asarray(edge_index[1], dtype=np.int64)
    deg = np.bincount(dst0, minlength=n).astype(np.float64) + 1.0
    dinv0 = (1.0 / np.sqrt(deg)).astype(np.float32)

    # permute nodes within each core: game nodes first, so layer-2
    # aggregation only needs the first G2 groups per core
    is_game = np.zeros(npad, dtype=bool)
    is_game[np.unique(np.asarray(game_indices, dtype=np.int64))] = True
    newpos = np.empty(npad, dtype=np.int64)
    gcount = []
    for c in range(nc):
        ids = np.arange(c * shard, (c + 1) * shard)
        gm = is_game[ids]
        gcount.append(int(gm.sum()))
        order = np.concatenate([ids[gm], ids[~gm]])
        newpos[order] = ids
    inv = np.empty(npad, dtype=np.int64)
    inv[newpos] = np.arange(npad)
    G2 = (max(gcount) + 127) // 128
    src = newpos[src0]
    dst = newpos[dst0]
    dinv_pad_full = np.ones(npad, dtype=np.float32)
    dinv_pad_full[newpos[:n]] = dinv0

    G = shard // 128                       # natural groups per core
    ncoh = (G + coh - 1) // coh
    # uneven chunks: small chunk 0 so AllGather-0 (and the first gathers)
    # start early; max 30 tiles keeps chunk tables < 32768 rows (int16)
    sizes = [10, 30, 29, 29]
    assert sum(sizes) == G and len(sizes) == nchunk
    ch_a = []
    lo = 0
    for s_ in sizes:
        ch_a.append((lo, lo + s_))
        lo += s_
    rows_a = [nc * (hi - lo) * 128 for lo, hi in ch_a]
    assert max(rows_a) <= 32767

    tile_q = np.zeros(G, dtype=np.int64)
    for q, (lo, hi) in enumerate(ch_a):
        tile_q[lo:hi] = q
    nodes = np.arange(npad, dtype=np.int64)
    c_of = nodes // shard
    loc = nodes % shard
    j_of = loc // 128
    p_of = loc % 128
    q_of = tile_q[j_of]                    # chunk of node (both layers)
    lo_a = np.array([lo for lo, hi in ch_a], dtype=np.int64)[q_of]
    nt_a = np.array([hi - lo for lo, hi in ch_a], dtype=np.int64)[q_of]
    row_in_chunk = c_of * nt_a * 128 + (j_of - lo_a) * 128 + p_of

    owner = dst // shard
    # per-core edge lists sorted by (group, chunk); self loops excluded
    core_el = []
    e_cgq = np.zeros((nc, G, nchunk), dtype=np.int64)
    for c in range(nc):
        m = owner == c
        s_, d_ = src[m], dst[m]
        g_ = (d_ - c * shard) // 128
        qe = q_of[s_]
        key = g_ * nchunk + qe
        order = np.argsort(key, kind="stable")
        ko = key[order]
        rows_o = row_in_chunk[s_][order]
        dpos_o = (d_[order] % 128).astype(np.int64)
        starts = np.searchsorted(ko, np.arange(G * nchunk))
        ends = np.searchsorted(ko, np.arange(G * nchunk) + 1)
        el = {}
        for g in range(G):
            for q in range(nchunk):
                b = g * nchunk + q
                a_, b_ = int(starts[b]), int(ends[b])
                el[(g, q)] = (rows_o[a_:b_], dpos_o[a_:b_])
                e_cgq[c, g, q] = b_ - a_
        core_el.append(el)

    C_gq = e_cgq.max(axis=0)               # common per-(g,q) capacity

    def build_set(G_eff):
        """Common layout + per-core idx/dloc over groups [0, G_eff)."""
        ncoh_e = (G_eff + coh - 1) // coh
        call_meta = []
        grp_meta = [[] for _ in range(G_eff)]
        oh_tiles = [0] * G_eff
        col_base = 0
        rr = 0
        for h in range(ncoh_e):
            gs = list(range(h * coh, min((h + 1) * coh, G_eff)))
            meta_h = []
            buf_off = 0
            for q in range(nchunk):
                R = int(C_gq[gs, q].sum())
                T = (R + 127) // 128
                if T == 0:
                    continue
                off = 0
                for g in gs:
                    cgq = int(C_gq[g, q])
                    if cgq > 0:
                        ta, tb = off // 128, (off + cgq - 1) // 128
                        grp_meta[g].append((q, off, cgq, buf_off + ta,
                                            tb - ta + 1))
                        oh_tiles[g] += tb - ta + 1
                    off += cgq
                t0 = 0
                while t0 < T:
                    tp = min(cap_tiles, T - t0)
                    meta_h.append((rr % nq, col_base + t0 * 8, tp * 8, tp,
                                   buf_off + t0))
                    rr += 1
                    t0 += tp
                col_base += T * 8
                buf_off += T
            call_meta.append(meta_h)
        total_tiles = col_base // 8
        n_oh = sum(oh_tiles)
        colq = {}
        for m_h in call_meta:
            for (qn, col_lo, ncols, ntq, off) in m_h:
                pass
        # colq per call col_lo -> chunk q: recover from layout order
        colq = {}
        col = 0
        for h in range(ncoh_e):
            gs = list(range(h * coh, min((h + 1) * coh, G_eff)))
            for q in range(nchunk):
                R = int(C_gq[gs, q].sum())
                T = (R + 127) // 128
                for t0 in range(0, T, cap_tiles):
                    colq[col + t0 * 8] = q
                col += T * 8
        cores = []
        for c in range(nc):
            el = core_el[c]
            idx16 = np.zeros(total_tiles * 128, dtype=np.int16)
            dloc = np.full((n_oh, 128), 300.0, dtype=np.float16)
            ohc = 0
            tile_base = 0
            for h in range(ncoh_e):
                gs = list(range(h * coh, min((h + 1) * coh, G_eff)))
                for q in range(nchunk):
                    R = int(C_gq[gs, q].sum())
                    T = (R + 127) // 128
                    if T == 0:
                        continue
                    seg = np.zeros(T * 128, dtype=np.int16)
                    off = 0
                    for g in gs:
                        rows_e, dpos_e = el[(g, q)]
                        ne = rows_e.shape[0]
                        seg[off:off + ne] = rows_e.astype(np.int16)
                        off += int(C_gq[g, q])
                    idx16[tile_base * 128:(tile_base + T) * 128] = seg
                    tile_base += T
            for g in range(G_eff):
                for (q, off, cgq, mt0, ntl) in grp_meta[g]:
                    rows_e, dpos_e = el[(g, q)]
                    ne = rows_e.shape[0]
                    ta = off // 128
                    dl = np.full(ntl * 128, 300.0, dtype=np.float16)
                    s0 = off - ta * 128
                    dl[s0:s0 + ne] = dpos_e
                    dloc[ohc:ohc + ntl] = dl.reshape(ntl, 128)
                    ohc += ntl
            assert ohc == n_oh
            cores.append((_wrap_idx16(idx16),
                          np.ascontiguousarray(dloc.T)))
        return dict(call_meta=call_meta, grp_meta=grp_meta,
                    oh_tiles=oh_tiles, total_tiles=total_tiles,
                    n_oh=n_oh, colq=colq, G_eff=G_eff), cores

    set1, cores1 = build_set(G)
    set2, cores2 = build_set(G2)

    per_core = []
    for c in range(nc):
        jj = np.arange(shard)
        dinv_nat = dinv_pad_full[c * shard + jj].reshape(G, 128).T.copy()
        ddrow = np.broadcast_to(
            dinv_pad_full[c * shard + jj].astype(np.float16)[None, :],
            (128, shard)).copy()
        old = inv[c * shard + jj]
        xs_shape = np.zeros((shard, x.shape[1]), dtype=np.float32)
        m_ = old < n
        xs_shape[m_] = x[old[m_]]
        xT = np.ascontiguousarray(xs_shape.T).astype(np.float16)
        per_core.append(dict(idx1=cores1[c][0], dloc1=cores1[c][1],
                             idx2=cores2[c][0], dloc2=cores2[c][1],
                             dinv_nat=dinv_nat, ddrow=ddrow, xT=xT))

    meta = dict(ch_a=ch_a, rows_a=rows_a, set1=set1, set2=set2,
                newpos=newpos, G2=G2)
    return per_core, meta, G


def _fold_bn(gamma, beta, mean, var, b):
    s = (gamma / np.sqrt(var + EPS)).astype(np.float32)
    t = ((b - mean) * s + beta).astype(np.float32)
    return s.reshape(H, 1), t.reshape(H, 1)


# ---------------------------------------------------------------- bass build
def _build(cfg, G, meta):
    import concourse.bacc as bacc
    import concourse.bass as bass
    import concourse.mybir as mybir
    import concourse.tile as tile

    fp32 = mybir.dt.float32
    fp16 = mybir.dt.float16
    i16 = mybir.dt.int16
    AF = mybir.ActivationFunctionType

    nc_ = cfg["NC"]
    shard = cfg["SHARD"]
    h = cfg["H"]
    fin = cfg["F_IN"]
    nchunk = cfg["NCHUNK"]
    coh = cfg["COH"]
    ch_a = meta["ch_a"]
    rows_a = meta["rows_a"]
    set1 = meta["set1"]
    set2 = meta["set2"]
    G2 = meta["G2"]
    ncoh = len(set1["call_meta"])
    Tg_max = max(max(set1["oh_tiles"]), max(set2["oh_tiles"]))

    nc = bacc.Bacc(None, target_bir_lowering=False, debug=False,
                   num_devices=nc_, num_swdge_queues=cfg["NQ"])

    iota_in = nc.dram_tensor("iota", [128, Tg_max * 128], fp16,
                             kind="ExternalInput")
    dloc_in = nc.dram_tensor("dloc1", [128, set1["n_oh"]], fp16,
                             kind="ExternalInput")
    idx_in = nc.dram_tensor("idx1", [128, set1["total_tiles"] * 8], i16,
                            kind="ExternalInput")
    dloc2_in = nc.dram_tensor("dloc2", [128, set2["n_oh"]], fp16,
                              kind="ExternalInput")
    idx2_in = nc.dram_tensor("idx2", [128, set2["total_tiles"] * 8], i16,
                             kind="ExternalInput")
    iden_in = nc.dram_tensor("iden", [128, 128], fp16, kind="ExternalInput")
    ddrow_in = nc.dram_tensor("ddrow", [128, shard], fp16,
                              kind="ExternalInput")
    xT_in = nc.dram_tensor("xT", [fin, shard], fp16, kind="ExternalInput")
    w1_in = nc.dram_tensor("W1", [fin, h], fp16, kind="ExternalInput")
    w2_in = nc.dram_tensor("W2", [h, h], fp16, kind="ExternalInput")
    wf_in = nc.dram_tensor("Wf", [h, 2], fp16, kind="ExternalInput")
    bf_in = nc.dram_tensor("bf_rep", [128, 2], fp32, kind="ExternalInput")
    s1_in = nc.dram_tensor("s1", [h, 1], fp32, kind="ExternalInput")
    t1_in = nc.dram_tensor("t1", [h, 1], fp32, kind="ExternalInput")
    s2_in = nc.dram_tensor("s2", [h, 1], fp32, kind="ExternalInput")
    t2_in = nc.dram_tensor("t2", [h, 1], fp32, kind="ExternalInput")
    dn_in = nc.dram_tensor("dinv_nat", [128, G], fp32, kind="ExternalInput")
    out_lp = nc.dram_tensor("logp", [128, 2 * G2], fp32,
                            kind="ExternalOutput")

    with tile.TileContext(nc) as tc:
        with (
            tc.tile_pool(name="res", bufs=1) as res,
            tc.tile_pool(name="big", bufs=1) as big,
            tc.tile_pool(name="stream", bufs=1) as st,
            tc.tile_pool(name="ps", bufs=1, space="PSUM") as ps,
            tc.tile_pool(name="dram", bufs=1, space="DRAM") as dram,
        ):
            iota_t = res.tile([128, Tg_max, 128], fp16)
            dloc_t = res.tile([128, set1["n_oh"]], fp16)
            idx_t = res.tile([128, set1["total_tiles"] * 8], i16)
            dloc2_t = res.tile([128, set2["n_oh"]], fp16)
            idx2_t = res.tile([128, set2["total_tiles"] * 8], i16)
            iden_t = res.tile([128, 128], fp16)
            ddrow_t = res.tile([128, shard], fp16)
            w1_t = res.tile([fin, h], fp16)
            w2_t = res.tile([h, h], fp16)
            wf_t = res.tile([h, 2], fp16)
            bf_t = res.tile([128, 2], fp32)
            s1_t = res.tile([h, 1], fp32)
            t1_t = res.tile([h, 1], fp32)
            s2_t = res.tile([h, 1], fp32)
            t2_t = res.tile([h, 1], fp32)
            dn_t = res.tile([128, G], fp32)
            def make_ohcol(S):
                oc = [0] * S["G_eff"]
                acc = 0
                for g in range(S["G_eff"]):
                    oc[g] = acc
                    acc += S["oh_tiles"][g]
                return oc

            ohcol = make_ohcol(set1)
            ohcol2 = make_ohcol(set2)
            call_meta = set1["call_meta"]
            oh_tiles = set1["oh_tiles"]

            for t_, i_ in ((iden_t, iden_in), (w1_t, w1_in), (w2_t, w2_in),
                           (wf_t, wf_in), (bf_t, bf_in), (s1_t, s1_in),
                           (t1_t, t1_in), (s2_t, s2_in), (t2_t, t2_in),
                           (dn_t, dn_in)):
                nc.sync.dma_start(out=t_[:], in_=i_[:])
            nc.sync.dma_start(out=iota_t[:],
                              in_=iota_in[:].rearrange("p (k d) -> p k d",
                                                       d=128))

            # progressive idx/dloc loads: piece 0 lands before the first
            # gathers; later pieces stream behind stage A's chunk loop
            cb = [0] * (ncoh + 1)
            ob = [0] * (ncoh + 1)
            for h_ in range(ncoh):
                ce = cb[h_]
                for (qn, col_lo, ncols, ntq, off) in call_meta[h_]:
                    ce = max(ce, col_lo + ncols)
                cb[h_ + 1] = ce
                ge = min((h_ + 1) * coh, G)
                ob[h_ + 1] = ohcol[ge - 1] + oh_tiles[ge - 1]
            pieces = [(0, 1)] + [(a, b) for a, b in
                                 zip([1, 9, 17], [9, 17, ncoh])]

            def load_piece(k):
                hlo, hhi = pieces[k]
                c0, c1 = cb[hlo], cb[hhi]
                o0, o1 = ob[hlo], ob[hhi]
                if c1 > c0:
                    nc.sync.dma_start(out=idx_t[:, c0:c1],
                                      in_=idx_in[:, c0:c1])
                if o1 > o0:
                    nc.sync.dma_start(out=dloc_t[:, o0:o1],
                                      in_=dloc_in[:, o0:o1])

            load_piece(0)
            nc.sync.dma_start(out=ddrow_t[:], in_=ddrow_in[:])

            nc.sync.dma_start(out=idx2_t[:], in_=idx2_in[:])
            nc.sync.dma_start(out=dloc2_t[:], in_=dloc2_in[:])

            def edge_layer(S, ixt, dlt, oc, tables, xself, s_t, t_t,
                           post_group):
                cm, ot, gm, cq, Ge = (S["call_meta"], S["oh_tiles"],
                                      S["grp_meta"], S["colq"], S["G_eff"])
                for hcoh in range(len(cm)):
                    gs = list(range(hcoh * coh, min((hcoh + 1) * coh, Ge)))
                    T_h = sum(m[3] for m in cm[hcoh])
                    msg = st.tile([128, max(T_h, 1), h], fp16, name="msg",
                                  tag="msg", bufs=2)
                    for (qn, col_lo, ncols, ntq, off) in cm[hcoh]:
                        nidx = ntq * 128
                        nc.gpsimd.dma_gather(
                            msg[:, off:off + ntq, :],
                            tables[cq[col_lo]],
                            ixt[:, col_lo:col_lo + ncols],
                            nidx,
                            nidx,
                            h,
                            queue_num=qn,
                        )
                    for g in gs:
                        Tg = ot[g]
                        oh = st.tile([128, Tg_max, 128], fp16, name="oh",
                                     tag="oh", bufs=3)
                        if Tg > 0:
                            nc.vector.tensor_tensor(
                                out=oh[:, :Tg, :],
                                in0=iota_t[:, :Tg, :],
                                in1=dlt[:, oc[g]:oc[g] + Tg]
                                    .to_broadcast([128, Tg, 128]),
                                op=mybir.AluOpType.is_equal,
                            )
                        pg = ps.tile([h, 128], fp32, name="pg", tag="pg",
                                     bufs=4)
                        # self-loop term: xs[tile g]^T via identity
                        nc.tensor.matmul(pg[:],
                                         xself[:, g * 128:(g + 1) * 128],
                                         iden_t[:],
                                         start=True, stop=(Tg == 0))
                        i = 0
                        for (q, off, cgq, mt0, ntl) in gm[g]:
                            for t in range(ntl):
                                nc.tensor.matmul(pg[:], msg[:, mt0 + t, :],
                                                 oh[:, i, :],
                                                 start=False,
                                                 stop=(i == Tg - 1))
                                i += 1
                        tmp = st.tile([h, 128], fp32, name="tmp", tag="tmp",
                                      bufs=4)
                        nc.vector.tensor_tensor(
                            out=tmp[:], in0=pg[:],
                            in1=ddrow_t[:, g * 128:(g + 1) * 128],
                            op=mybir.AluOpType.mult,
                        )
                        hblk = st.tile([h, 128], fp16, name="hblk",
                                       tag="hblk", bufs=4)
                        nc.scalar.activation(
                            out=hblk[:], in_=tmp[:],
                            func=AF.Relu, bias=t_t[:], scale=s_t[:],
                        )
                        post_group(g, hblk)

            # ---- stage A: xs1 compute, staged + AllGather'd per chunk
            xs1_shard = dram.tile([shard, h], fp16)
            xs1_q = [dram.tile([rows_a[q], h], fp16, addr_space="Shared",
                               name=f"xs1q{q}")
                     for q in range(nchunk)]
            xsb = big.tile([128, G * 128], fp16, name="xsb", tag="big_a")
            for q, (lo, hi) in enumerate(ch_a):
                for j in range(lo, hi):
                    lhsT = st.tile([128, 128], fp16, name="xTt",
                                   tag="lhsT", bufs=4)
                    nc.sync.dma_start(
                        out=lhsT[:], in_=xT_in[:, j * 128:(j + 1) * 128])
                    pxs = ps.tile([128, h], fp32, name="pxs", tag="pxs",
                                  bufs=2)
                    nc.tensor.matmul(pxs[:], lhsT[:], w1_t[:], start=True,
                                     stop=True)
                    nc.vector.tensor_scalar(
                        out=xsb[:, j * 128:(j + 1) * 128], in0=pxs[:],
                        scalar1=dn_t[:, j:j + 1], scalar2=None,
                        op0=mybir.AluOpType.mult)
                rows = hi - lo
                dest = bass.AP(xs1_shard[:].tensor, lo * 128 * h,
                               [[h, 128], [128 * h, rows], [1, h]])
                nc.sync.dma_start(out=dest, in_=xsb[:].rearrange(
                    "p (j f) -> p j f", f=h)[:, lo:hi, :])
                nc.gpsimd.collective_compute(
                    "AllGather", mybir.AluOpType.bypass,
                    replica_groups=[list(range(nc_))],
                    ins=[xs1_shard[lo * 128:hi * 128, :].opt()],
                    outs=[xs1_q[q][:].opt()],
                )
                if q + 1 < len(pieces):
                    load_piece(q + 1)

            # ---- layer 1 with interleaved xs2 production + AG2
            xs2_shard = dram.tile([G * 128, h], fp16)
            xs2q_int = [dram.tile([rows_a[q], h], fp16,
                                  addr_space="Shared", name=f"xs2qi{q}")
                        for q in range(nchunk)]
            xs2b = big.tile([128, G * 128], fp16, name="xs2b", tag="big_c")
            g_last = {hi - 1: q for q, (lo, hi) in enumerate(ch_a)}

            def post_group_a(g, hblk):
                pxs = ps.tile([128, h], fp32, name="pxs2", tag="pxs",
                              bufs=2)
                nc.tensor.matmul(pxs[:], hblk[:], w2_t[:], start=True,
                                 stop=True)
                nc.vector.tensor_scalar(
                    out=xs2b[:, g * 128:(g + 1) * 128], in0=pxs[:],
                    scalar1=dn_t[:, g:g + 1], scalar2=None,
                    op0=mybir.AluOpType.mult)
                if g in g_last:
                    q = g_last[g]
                    lo, hi = ch_a[q]
                    rows = hi - lo
                    dest = bass.AP(xs2_shard[:].tensor, lo * 128 * h,
                                   [[h, 128], [128 * h, rows], [1, h]])
                    nc.sync.dma_start(out=dest, in_=xs2b[:].rearrange(
                        "p (j f) -> p j f", f=h)[:, lo:hi, :])
                    nc.gpsimd.collective_compute(
                        "AllGather", mybir.AluOpType.bypass,
                        replica_groups=[list(range(nc_))],
                        ins=[xs2_shard[lo * 128:hi * 128, :].opt()],
                        outs=[xs2q_int[q][:].opt()],
                    )

            edge_layer(set1, idx_t[:], dloc_t[:], ohcol,
                       [t[:] for t in xs1_q], xsb[:], s1_t, t1_t,
                       post_group_a)

            lg = res.tile([128, 2 * G2], fp32)

            def post_group_b(g, hblk):
                plg = ps.tile([128, 2], fp32, name="plg", tag="plg",
                              bufs=2)
                nc.tensor.matmul(plg[:], hblk[:], wf_t[:], start=True,
                                 stop=True)
                nc.vector.tensor_add(out=lg[:, 2 * g:2 * g + 2],
                                     in0=plg[:], in1=bf_t[:])

            edge_layer(set2, idx2_t[:], dloc2_t[:], ohcol2,
                       [t[:] for t in xs2q_int], xs2b[:], s2_t, t2_t,
                       post_group_b)

            def strided(base, start):
                a = base[:]
                return bass.AP(a.tensor, a.offset + start,
                               [a.ap[0], [2, G2]])

            z0, z1 = strided(lg, 0), strided(lg, 1)
            mx = res.tile([128, G2], fp32)
            nc.vector.tensor_tensor(out=mx[:], in0=z0, in1=z1,
                                    op=mybir.AluOpType.max)
            sm0 = res.tile([128, G2], fp32)
            sm1 = res.tile([128, G2], fp32)
            nc.vector.tensor_sub(out=sm0[:], in0=z0, in1=mx[:])
            nc.vector.tensor_sub(out=sm1[:], in0=z1, in1=mx[:])
            e0 = res.tile([128, G2], fp32)
            e1 = res.tile([128, G2], fp32)
            nc.scalar.activation(out=e0[:], in_=sm0[:], func=AF.Exp)
            nc.scalar.activation(out=e1[:], in_=sm1[:], func=AF.Exp)
            se = res.tile([128, G2], fp32)
            nc.vector.tensor_add(out=se[:], in0=e0[:], in1=e1[:])
            ls = res.tile([128, G2], fp32)
            nc.scalar.activation(out=ls[:], in_=se[:], func=AF.Ln)
            nc.vector.tensor_sub(out=sm0[:], in0=sm0[:], in1=ls[:])
            nc.vector.tensor_sub(out=sm1[:], in0=sm1[:], in1=ls[:])
            lpo = res.tile([128, 2 * G2], fp32)
            nc.vector.tensor_copy(out=strided(lpo, 0), in_=sm0[:])
            nc.vector.tensor_copy(out=strided(lpo, 1), in_=sm1[:])
            nc.sync.dma_start(out=out_lp[:], in_=lpo[:])

    nc.compile()
    return nc


# ---------------------------------------------------------------- main entry
def _run(x, edge_index, game_indices,
         W1, b1, g1, be1, m1, v1, W2, b2, g2, be2, m2, v2, Wf, bf,
         trace=False, cfg=None):
    from concourse import bass_utils

    if cfg is None:
        cfg = dict(N=N, NPAD=NPAD, SHARD=SHARD, NC=NC, H=H, F_IN=F_IN,
                   NCHUNK=NCHUNK, COH=COH, NQ=NQ, NIDX_CAP=1024)

    x = np.asarray(x, dtype=np.float32)
    key = ("prep", x.shape, int(np.asarray(edge_index)[0, 0]),
           int(np.asarray(edge_index).sum() % (1 << 31)))
    if key in _CACHE:
        per_core, meta, G = _CACHE[key]
    else:
        per_core, meta, G = _prepare(x, np.asarray(edge_index),
                                     game_indices, cfg)
        _CACHE.clear()
        _CACHE[key] = (per_core, meta, G)

    bkey = ("bass", G, meta["G2"],
            meta["set1"]["total_tiles"], meta["set2"]["total_tiles"],
            tuple(tuple(m) for h_ in meta["set1"]["call_meta"] for m in h_),
            tuple(tuple(m) for h_ in meta["set2"]["call_meta"] for m in h_))
    if bkey in _CACHE:
        nc_m = _CACHE[bkey]
    else:
        nc_m = _build(cfg, G, meta)
        _CACHE[bkey] = nc_m

    s1, t1 = _fold_bn(np.asarray(g1), np.asarray(be1), np.asarray(m1),
                      np.asarray(v1), np.asarray(b1))
    s2, t2 = _fold_bn(np.asarray(g2), np.asarray(be2), np.asarray(m2),
                      np.asarray(v2), np.asarray(b2))
    Tg_max = max(max(meta["set1"]["oh_tiles"]),
                 max(meta["set2"]["oh_tiles"]))
    iota = np.tile(np.arange(128, dtype=np.float16), (128, Tg_max))
    iden = np.eye(128, dtype=np.float16)
    bf_rep = np.broadcast_to(np.asarray(bf, dtype=np.float32), (128, 2)).copy()

    ncores = cfg["NC"]
    in_maps = []
    for c in range(ncores):
        pc = per_core[c]
        in_maps.append(dict(
            xT=pc["xT"], W1=np.asarray(W1, np.float16),
            W2=np.asarray(W2, np.float16), Wf=np.asarray(Wf, np.float16),
            bf_rep=bf_rep, s1=s1, t1=t1, s2=s2, t2=t2, iota=iota,
            iden=iden, idx1=pc["idx1"], dloc1=pc["dloc1"],
            idx2=pc["idx2"], dloc2=pc["dloc2"],
            ddrow=pc["ddrow"], dinv_nat=pc["dinv_nat"],
        ))
    res = bass_utils.run_bass_kernel_spmd(
        nc_m, in_maps, core_ids=list(range(ncores)), trace=trace)

    class _Res:
        pass

    r = _Res()
    r.results = res.results
    r.exec_time_ns = res.exec_time_ns
    r.parts = (res,)

    gi = meta["newpos"][np.asarray(game_indices, dtype=np.int64)]
    shard = cfg["SHARD"]
    ci = gi // shard
    gidx = (gi % shard) // 128
    pi = gi % 128
    assert gidx.max() < meta["G2"]
    lp = np.stack([res.results[c]["logp"] for c in range(ncores)])
    out = np.empty((gi.shape[0], 2), dtype=np.float32)
    out[:, 0] = lp[ci, pi, 2 * gidx]
    out[:, 1] = lp[ci, pi, 2 * gidx + 1]
    return out, r


def kernel(**inputs):
    out, _ = _run(**inputs)
    return out


def kernel_profiled(**inputs):
    out, res = _run(**inputs, trace=True)
    return out, res
